# revision 10
# baseline (speedup 1.0000x reference)
"""Trainium2 Bass kernel for the dense RandLA-Net block.

Reference computation (per batch b, point n, K=16 neighbors):
    enc   = [center(3), npos(3), rel(3), dist(1)]            # 10 dims
    rp    = relu(enc @ W_rel + b_rel)                        # 64
    f     = [rp, nfeat]                                      # 128
    att   = softmax_k(f @ W_att)                             # 128
    agg   = sum_k f * att                                    # 128
    out   = relu(agg @ W_glob + b_glob)                      # 128

Sharding: 8 cores = 4 batches x 2 point-halves (8192 points/core).

Host prep lays the per-(point, neighbor) token streams out in the
column order the kernel consumes (tile of 512 points x 16 k-slabs):
xg holds the 64 gathered feature channels, pgd the gathered neighbor
position and the center->neighbor distance.  On device, each tile
streams both into SBUF partitions 0:64 / 64:68 and runs the fused
pipeline per 512-column k-slab: rp is a 4-row matmul [npos;dist] plus
a 3-row center matmul folded via PSUM accumulation (rel is
algebraically folded:  Wc*center + Wn*npos + Wr*(npos-center) =
(Wc-Wr)*center + (Wn+Wr)*npos), then relu(+bias) lands rp on
partitions 64:128 next to nfeat, one 128x128 matmul produces the
attention scores, exp + multiply build the softmax numerator terms,
and two PSUM-accumulated matmuls against an un-permuting identity sum
over the 16 k-slabs.  The whole pipeline runs in the "swapped"
channel layout f = [nfeat(0:64); rp(64:128)], handled by a permuted
W_att and the roll-by-64 accumulation identity.
"""

import os
import sys

import numpy as np

sys.path.insert(0, "/opt/trn_rl_repo")

import ml_dtypes

import concourse.bass as bass
import concourse.tile as tile
from concourse import mybir, bacc
from concourse.bass_utils import run_bass_kernel_spmd

F32 = mybir.dt.float32
BF16 = mybir.dt.bfloat16
AF = mybir.ActivationFunctionType
OP = mybir.AluOpType
BF = ml_dtypes.bfloat16

B, C_IN, N, K = 4, 64, 16384, 16
D_REL, C_MID, C_OUT = 64, 128, 128
NP = N // 2            # points per core
PK = NP * K            # pair columns per core (131072)
NT = 16                # tiles (= point blocks of 512)
TC = 512               # points per tile
LT = TC * K            # 8192 cols per tile


def _build_kernel():
    nc = bacc.Bacc("TRN2", target_bir_lowering=False)

    xg = nc.dram_tensor("xg", [64, PK], BF16, kind="ExternalInput")
    pgd = nc.dram_tensor("pgd", [8, PK], BF16, kind="ExternalInput")
    wnd = nc.dram_tensor("wnd", [128, 64], BF16, kind="ExternalInput")
    wattsw = nc.dram_tensor("wattsw", [128, 128], BF16, kind="ExternalInput")
    pswap = nc.dram_tensor("pswap", [128, 128], BF16, kind="ExternalInput")
    wglob = nc.dram_tensor("wglob", [128, 128], BF16, kind="ExternalInput")
    bglob = nc.dram_tensor("bglob", [128, 1], F32, kind="ExternalInput")
    outp = nc.dram_tensor("outp", [128, NP], F32, kind="ExternalOutput")

    with tile.TileContext(nc) as tc:
        with tc.tile_pool(name="persist", bufs=1) as pp:
            wnd_sb = pp.tile([128, 64], BF16)
            wattsw_sb = pp.tile([128, 128], BF16)
            pswap_sb = pp.tile([128, 128], BF16)
            wglob_sb = pp.tile([128, 128], BF16)
            bglob_sb = pp.tile([128, 1], F32)
            nc.sync.dma_start(out=wnd_sb, in_=wnd.ap())
            nc.sync.dma_start(out=wattsw_sb, in_=wattsw.ap())
            nc.sync.dma_start(out=pswap_sb, in_=pswap.ap())
            nc.sync.dma_start(out=wglob_sb, in_=wglob.ap())
            nc.sync.dma_start(out=bglob_sb, in_=bglob.ap())

            with tc.tile_pool(name="g", bufs=4) as gp, \
                 tc.tile_pool(name="work", bufs=3) as wp, \
                 tc.tile_pool(name="accps", bufs=1, space="PSUM") as psa, \
                 tc.tile_pool(name="mps", bufs=3, space="PSUM") as mps:
                for t in range(NT):
                    tcols = slice(t * TC, (t + 1) * TC)
                    scols = slice(t * LT, (t + 1) * LT)
                    g = gp.tile([128, LT], BF16, tag="g")
                    nc.sync.dma_start(out=g[0:64, :], in_=xg.ap()[:, scols])
                    nc.sync.dma_start(out=g[64:72, :], in_=pgd.ap()[:, scols])
                    ps_den = psa.tile([128, 512], F32, tag="den")
                    ps_num = psa.tile([128, 512], F32, tag="num")
                    eu_prev = None
                    eup_prev = None
                    for cc in range(16):
                        ccols = slice(cc * 512, (cc + 1) * 512)
                        ps_rp = mps.tile([128, 512], F32, tag="rp")
                        nc.tensor.matmul(ps_rp[64:128, :],
                                         wnd_sb[64:72, :],
                                         g[64:72, ccols],
                                         start=True, stop=True,
                                         tile_position=(64, 64),
                                         skip_group_check=True)
                        if cc % 3 != 0:
                            nc.scalar.activation(out=g[64:128, ccols],
                                                 in_=ps_rp[64:128, :],
                                                 func=AF.Relu)
                        else:
                            nc.vector.tensor_scalar_max(out=g[64:128, ccols],
                                                        in0=ps_rp[64:128, :],
                                                        scalar1=0.0)
                        ps_s = mps.tile([128, 512], F32, tag="sc")
                        nc.tensor.matmul(ps_s, wattsw_sb, g[:, ccols],
                                         start=True, stop=True)
                        eu = wp.tile([128, 1024], BF16, tag="eu")
                        nc.scalar.activation(out=eu[:, 0:512], in_=ps_s,
                                             func=AF.Exp)
                        nc.vector.tensor_mul(eu[:, 512:1024], g[:, ccols],
                                             eu[:, 0:512])
                        if cc % 2 == 1:
                            eup = wp.tile([128, 1024], BF16, tag="eup")
                            nc.vector.tensor_add(eup, eu_prev, eu)
                            if cc % 4 == 3:
                                euq = wp.tile([128, 1024], BF16, tag="euq")
                                nc.vector.tensor_add(euq, eup_prev, eup)
                                nc.tensor.matmul(ps_den, pswap_sb,
                                                 euq[:, 0:512],
                                                 start=(cc == 3),
                                                 stop=(cc == 15),
                                                 skip_group_check=True)
                                nc.tensor.matmul(ps_num, pswap_sb,
                                                 euq[:, 512:1024],
                                                 start=(cc == 3),
                                                 stop=(cc == 15),
                                                 skip_group_check=True)
                            eup_prev = eup
                        eu_prev = eu
                    rcp = wp.tile([128, 512], F32, tag="rcp")
                    nc.vector.reciprocal_approx_fast(rcp, ps_den)
                    agg = wp.tile([128, 512], BF16, tag="agg")
                    nc.vector.tensor_mul(agg, ps_num, rcp)
                    ps_o = psa.tile([128, 512], F32, tag="den")
                    nc.tensor.matmul(ps_o, wglob_sb, agg, start=True,
                                     stop=True)
                    osb = wp.tile([128, 512], F32, tag="osb")
                    nc.scalar.activation(out=osb, in_=ps_o, func=AF.Relu,
                                         bias=bglob_sb, scale=1.0)
                    nc.sync.dma_start(out=outp.ap()[:, tcols], in_=osb)
    nc.compile()
    return nc


_NC = None


def _get_nc():
    global _NC
    if _NC is None:
        _NC = _build_kernel()
    return _NC


def _prep_core(core, x, pos, neigh, Wc, Wn, Wr, wd, W_att, W_glob, b_rel, b_glob):
    b = core // 2
    half = core % 2
    P0 = half * NP
    nb = neigh[b][P0:P0 + NP].astype(np.int64)      # [NP, K]
    xb = x[b]                                        # [64, N] f32
    posb = pos[b]                                    # [N, 3] f32

    # pair column c = t*8192 + k*512 + i -> (point n = P0 + t*512 + i, k)
    c = np.arange(PK)
    t_ = c >> 13
    k_ = (c >> 9) & 15
    i_ = c & 511
    n_ = t_ * TC + i_
    src = nb[n_, k_]                                 # neighbor point ids [PK]

    xg = xb[:, src].astype(BF)                       # [64, PK]
    npos = posb[src]                                 # [PK, 3] f32
    cen = posb[P0 + n_]                              # [PK, 3] f32
    dist = np.sqrt(((npos.astype(BF).astype(np.float32)
                     - cen.astype(BF).astype(np.float32)) ** 2).sum(1))
    pgd = np.empty((8, PK), dtype=BF)
    pgd[0:3] = npos.T.astype(BF)
    pgd[3] = dist.astype(BF)
    pgd[4:7] = cen.T.astype(BF)
    pgd[7] = 1.0

    wnd_h = np.zeros((128, 64), dtype=BF)
    wnd_h[64:67] = (Wn + Wr).astype(BF)
    wnd_h[67] = wd[0].astype(BF)
    wnd_h[68:71] = (Wc - Wr).astype(BF)
    wnd_h[71] = b_rel.astype(BF)

    perm = (np.arange(128) + 64) % 128

    return {
        "xg": np.ascontiguousarray(xg), "pgd": pgd,
        "wnd": wnd_h,
        "wattsw": W_att[np.ix_(perm, perm)].astype(BF),
        "pswap": np.roll(np.eye(128, dtype=np.float32), 64, axis=0).astype(BF),
        "wglob": W_glob.astype(BF),
        "bglob": b_glob.reshape(128, 1).astype(np.float32),
    }


def kernel(x, pos, neigh_idx, W_rel, b_rel, W_att, W_glob, b_glob, **kw):
    x = np.ascontiguousarray(np.asarray(x, dtype=np.float32))
    pos = np.ascontiguousarray(np.asarray(pos, dtype=np.float32))
    neigh = np.asarray(neigh_idx)
    W_rel = np.asarray(W_rel, dtype=np.float32)
    W_att = np.asarray(W_att, dtype=np.float32)
    W_glob = np.asarray(W_glob, dtype=np.float32)
    b_rel = np.asarray(b_rel, dtype=np.float32)
    b_glob = np.asarray(b_glob, dtype=np.float32)
    Wc, Wn, Wr, wd = W_rel[0:3], W_rel[3:6], W_rel[6:9], W_rel[9:10]

    nc = _get_nc()
    in_maps = [
        _prep_core(core, x, pos, neigh, Wc, Wn, Wr, wd, W_att, W_glob, b_rel, b_glob)
        for core in range(8)
    ]
    res = run_bass_kernel_spmd(nc, in_maps, core_ids=list(range(8)))
    out = np.zeros((B, C_OUT, N), np.float32)
    for core in range(8):
        b = core // 2
        P0 = (core % 2) * NP
        out[b, :, P0:P0 + NP] = res.results[core]["outp"]
    return out


# revision 11
# speedup vs baseline: 1.0863x; 1.0863x over previous
"""Trainium2 Bass kernel for the dense RandLA-Net block.

Reference computation (per batch b, point n, K=16 neighbors):
    enc   = [center(3), npos(3), rel(3), dist(1)]            # 10 dims
    rp    = relu(enc @ W_rel + b_rel)                        # 64
    f     = [rp, nfeat]                                      # 128
    att   = softmax_k(f @ W_att)                             # 128
    agg   = sum_k f * att                                    # 128
    out   = relu(agg @ W_glob + b_glob)                      # 128

Sharding: 8 cores = 4 batches x 2 point-halves (8192 points/core).

Host prep lays the per-(point, neighbor) token streams out in the
column order the kernel consumes (tile of 512 points x 16 k-slabs):
xg holds the 64 gathered feature channels, pgd the gathered neighbor
position and the center->neighbor distance.  On device, each tile
streams both into SBUF partitions 0:64 / 64:68 and runs the fused
pipeline per 512-column k-slab: rp is a 4-row matmul [npos;dist] plus
a 3-row center matmul folded via PSUM accumulation (rel is
algebraically folded:  Wc*center + Wn*npos + Wr*(npos-center) =
(Wc-Wr)*center + (Wn+Wr)*npos), then relu(+bias) lands rp on
partitions 64:128 next to nfeat, one 128x128 matmul produces the
attention scores, exp + multiply build the softmax numerator terms,
and two PSUM-accumulated matmuls against an un-permuting identity sum
over the 16 k-slabs.  The whole pipeline runs in the "swapped"
channel layout f = [nfeat(0:64); rp(64:128)], handled by a permuted
W_att and the roll-by-64 accumulation identity.
"""

import os
import sys

import numpy as np

sys.path.insert(0, "/opt/trn_rl_repo")

import ml_dtypes

import concourse.bass as bass
import concourse.tile as tile
from concourse import mybir, bacc
from concourse.bass_utils import run_bass_kernel_spmd

F32 = mybir.dt.float32
BF16 = mybir.dt.bfloat16
AF = mybir.ActivationFunctionType
OP = mybir.AluOpType
BF = ml_dtypes.bfloat16

B, C_IN, N, K = 4, 64, 16384, 16
D_REL, C_MID, C_OUT = 64, 128, 128
NP = N // 2            # points per core
PK = NP * K            # pair columns per core (131072)
NT = 16                # tiles (= point blocks of 512)
TC = 512               # points per tile
LT = TC * K            # 8192 cols per tile


def _build_kernel():
    nc = bacc.Bacc("TRN2", target_bir_lowering=False)

    xg = nc.dram_tensor("xg", [64, PK], BF16, kind="ExternalInput")
    pgd = nc.dram_tensor("pgd", [8, PK], BF16, kind="ExternalInput")
    wnd = nc.dram_tensor("wnd", [128, 64], BF16, kind="ExternalInput")
    wattsw = nc.dram_tensor("wattsw", [128, 128], BF16, kind="ExternalInput")
    pswap = nc.dram_tensor("pswap", [128, 128], BF16, kind="ExternalInput")
    wglob = nc.dram_tensor("wglob", [128, 128], BF16, kind="ExternalInput")
    bglob = nc.dram_tensor("bglob", [128, 1], F32, kind="ExternalInput")
    outp = nc.dram_tensor("outp", [128, NP], F32, kind="ExternalOutput")

    with tile.TileContext(nc) as tc:
        with tc.tile_pool(name="persist", bufs=1) as pp:
            wnd_sb = pp.tile([128, 64], BF16)
            wattsw_sb = pp.tile([128, 128], BF16)
            pswap_sb = pp.tile([128, 128], BF16)
            wglob_sb = pp.tile([128, 128], BF16)
            bglob_sb = pp.tile([128, 1], F32)
            nc.sync.dma_start(out=wnd_sb, in_=wnd.ap())
            nc.sync.dma_start(out=wattsw_sb, in_=wattsw.ap())
            nc.sync.dma_start(out=pswap_sb, in_=pswap.ap())
            nc.sync.dma_start(out=wglob_sb, in_=wglob.ap())
            nc.sync.dma_start(out=bglob_sb, in_=bglob.ap())

            with tc.tile_pool(name="g", bufs=3) as gp, \
                 tc.tile_pool(name="work", bufs=2) as wp, \
                 tc.tile_pool(name="accps", bufs=1, space="PSUM") as psa, \
                 tc.tile_pool(name="mps", bufs=3, space="PSUM") as mps:
                for t in range(NT):
                    tcols = slice(t * TC, (t + 1) * TC)
                    scols = slice(t * LT, (t + 1) * LT)
                    g = gp.tile([128, LT], BF16, tag="g")
                    nc.sync.dma_start(out=g[0:64, :], in_=xg.ap()[:, scols])
                    nc.sync.dma_start(out=g[64:72, :], in_=pgd.ap()[:, scols])
                    ps_den = psa.tile([128, 512], F32, tag="den")
                    ps_num = psa.tile([128, 512], F32, tag="num")
                    eu_prev = None
                    eup_prev = None
                    for cc in range(16):
                        ccols = slice(cc * 512, (cc + 1) * 512)
                        ps_rp = mps.tile([128, 512], F32, tag="rp")
                        nc.tensor.matmul(ps_rp[64:128, :],
                                         wnd_sb[64:72, :],
                                         g[64:72, ccols],
                                         start=True, stop=True,
                                         tile_position=(64, 64),
                                         skip_group_check=True)
                        if cc % 3 != 0:
                            nc.scalar.activation(out=g[64:128, ccols],
                                                 in_=ps_rp[64:128, :],
                                                 func=AF.Relu)
                        else:
                            nc.vector.tensor_scalar_max(out=g[64:128, ccols],
                                                        in0=ps_rp[64:128, :],
                                                        scalar1=0.0)
                        ps_s = mps.tile([128, 512], F32, tag="sc")
                        nc.tensor.matmul(ps_s, wattsw_sb, g[:, ccols],
                                         start=True, stop=True)
                        eu = wp.tile([128, 1024], BF16, tag="eu")
                        nc.scalar.activation(out=eu[:, 0:512], in_=ps_s,
                                             func=AF.Exp)
                        nc.vector.tensor_mul(eu[:, 512:1024], g[:, ccols],
                                             eu[:, 0:512])
                        if cc % 2 == 1:
                            eup = wp.tile([128, 1024], BF16, tag="eup")
                            nc.vector.tensor_add(eup, eu_prev, eu)
                            if cc % 4 == 3:
                                euq = wp.tile([128, 1024], BF16, tag="euq")
                                nc.vector.tensor_add(euq, eup_prev, eup)
                                nc.tensor.matmul(ps_den, pswap_sb,
                                                 euq[:, 0:512],
                                                 start=(cc == 3),
                                                 stop=(cc == 15),
                                                 skip_group_check=True)
                                nc.tensor.matmul(ps_num, pswap_sb,
                                                 euq[:, 512:1024],
                                                 start=(cc == 3),
                                                 stop=(cc == 15),
                                                 skip_group_check=True)
                            eup_prev = eup
                        eu_prev = eu
                    rcp = wp.tile([128, 512], F32, tag="rcp")
                    nc.vector.reciprocal_approx_fast(rcp, ps_den)
                    agg = wp.tile([128, 512], BF16, tag="agg")
                    nc.vector.tensor_mul(agg, ps_num, rcp)
                    ps_o = psa.tile([128, 512], F32, tag="den")
                    nc.tensor.matmul(ps_o, wglob_sb, agg, start=True,
                                     stop=True)
                    osb = wp.tile([128, 512], F32, tag="osb")
                    nc.scalar.activation(out=osb, in_=ps_o, func=AF.Relu,
                                         bias=bglob_sb, scale=1.0)
                    nc.sync.dma_start(out=outp.ap()[:, tcols], in_=osb)
    nc.compile()
    return nc


_NC = None


def _get_nc():
    global _NC
    if _NC is None:
        _NC = _build_kernel()
    return _NC


def _prep_core(core, x, pos, neigh, Wc, Wn, Wr, wd, W_att, W_glob, b_rel, b_glob):
    b = core // 2
    half = core % 2
    P0 = half * NP
    nb = neigh[b][P0:P0 + NP].astype(np.int64)      # [NP, K]
    xb = x[b]                                        # [64, N] f32
    posb = pos[b]                                    # [N, 3] f32

    # pair column c = t*8192 + k*512 + i -> (point n = P0 + t*512 + i, k)
    c = np.arange(PK)
    t_ = c >> 13
    k_ = (c >> 9) & 15
    i_ = c & 511
    n_ = t_ * TC + i_
    src = nb[n_, k_]                                 # neighbor point ids [PK]

    xg = xb[:, src].astype(BF)                       # [64, PK]
    npos = posb[src]                                 # [PK, 3] f32
    cen = posb[P0 + n_]                              # [PK, 3] f32
    dist = np.sqrt(((npos.astype(BF).astype(np.float32)
                     - cen.astype(BF).astype(np.float32)) ** 2).sum(1))
    pgd = np.empty((8, PK), dtype=BF)
    pgd[0:3] = npos.T.astype(BF)
    pgd[3] = dist.astype(BF)
    pgd[4:7] = cen.T.astype(BF)
    pgd[7] = 1.0

    wnd_h = np.zeros((128, 64), dtype=BF)
    wnd_h[64:67] = (Wn + Wr).astype(BF)
    wnd_h[67] = wd[0].astype(BF)
    wnd_h[68:71] = (Wc - Wr).astype(BF)
    wnd_h[71] = b_rel.astype(BF)

    perm = (np.arange(128) + 64) % 128

    return {
        "xg": np.ascontiguousarray(xg), "pgd": pgd,
        "wnd": wnd_h,
        "wattsw": W_att[np.ix_(perm, perm)].astype(BF),
        "pswap": np.roll(np.eye(128, dtype=np.float32), 64, axis=0).astype(BF),
        "wglob": W_glob.astype(BF),
        "bglob": b_glob.reshape(128, 1).astype(np.float32),
    }


def kernel(x, pos, neigh_idx, W_rel, b_rel, W_att, W_glob, b_glob, **kw):
    x = np.ascontiguousarray(np.asarray(x, dtype=np.float32))
    pos = np.ascontiguousarray(np.asarray(pos, dtype=np.float32))
    neigh = np.asarray(neigh_idx)
    W_rel = np.asarray(W_rel, dtype=np.float32)
    W_att = np.asarray(W_att, dtype=np.float32)
    W_glob = np.asarray(W_glob, dtype=np.float32)
    b_rel = np.asarray(b_rel, dtype=np.float32)
    b_glob = np.asarray(b_glob, dtype=np.float32)
    Wc, Wn, Wr, wd = W_rel[0:3], W_rel[3:6], W_rel[6:9], W_rel[9:10]

    nc = _get_nc()
    in_maps = [
        _prep_core(core, x, pos, neigh, Wc, Wn, Wr, wd, W_att, W_glob, b_rel, b_glob)
        for core in range(8)
    ]
    res = run_bass_kernel_spmd(nc, in_maps, core_ids=list(range(8)))
    out = np.zeros((B, C_OUT, N), np.float32)
    for core in range(8):
        b = core // 2
        P0 = (core % 2) * NP
        out[b, :, P0:P0 + NP] = res.results[core]["outp"]
    return out
